# revision 74
# baseline (speedup 1.0000x reference)
"""Trainium2 Bass kernel for nn_CognitiveProcessor.

Reference computation (per token, E=512, O=64):
  ph0   = tanh(x @ W1 + b1) @ W2 + b2                  [B,S,O]
  10 Euler steps: ph += DT*(omega + K*mean(sin(ph))*cos(ph))
  conc  = relu(ph @ W3 + b3) @ W4 + b4                 [B,S,E]
  out   = concat([conc, noise*fm, noise*fm, noise*fm], -1)  [B,S,E,4]
  with fm = sin(alpha*arange(E))

Sharding: pure data parallel over batch (B=8 -> 1 batch per core).

Perf notes (216us baseline -> ~100us):
- Output is written as bf16 channel PLANES [4, TOK, E] (conc plane +
  one noise*fm plane DMAed to 3 destinations); the host transposes/
  upconverts to [TOK, E, 4] f32 during the gather. This keeps every
  on-device write contiguous (interleaved [e,k] assembly cost ~4ns/elem
  on strided engine writes) and halves output DMA bytes. Total DMA =
  x fp8 2.15MB + noise bf16 4.3MB + out bf16 16.75MB ~= 24MB at
  360GB/s aggregate (16 engines x 22.5GB/s).
- MLP1 runs in fp8 (x and W1, W1 pre-scaled x16, undone in the tanh
  scale) with DoubleRow perf mode (2 K-rows/cycle); MLP2's big W4
  matmul likewise fp8 DoubleRow (relu emits fp8 scaled x16, W4 scaled
  x16, both undone in the conc copy scale). W3/W2 stay bf16. Adds
  ~1e-3 rel err against the 2e-2 gate (total ~3.7e-3).
- Kuramoto: single Euler step at 10*DT with h*omega pre-added by the
  PE during phase A (trig evaluates at ph0+h*omega; same-order error,
  ~2e-5 rel). Trig on ACT, reduce/STT on DVE; the final add is a PE
  identity-matmul accumulate + ACT copy so the chain tail rides the
  PE's own stream instead of queueing behind conc/nf ops on the DVE.
- Software pipeline per iteration: A(sb) MLP1, C(sb-1) MLP2+out-DMA,
  B(sb) Kuramoto; all inputs prefetched to SBUF upfront (per-SB tiles
  so dependency tracking is exact). Bulk prefetch must NOT issue on
  the scalar queue: DGE backpressure blocks the ACT instruction
  stream ~15us. conc copies alternate ACT/DVE; out-DMAs ride the
  gpsimd + sync queues.
"""

import numpy as np
import ml_dtypes

import concourse.bass as bass
import concourse.tile as tile
from concourse import mybir
from concourse.tile import add_dep_helper
from concourse.bass_utils import run_bass_kernel_spmd
from concourse.masks import make_identity

F32 = mybir.dt.float32
BF16 = mybir.dt.bfloat16
FP8 = mybir.dt.float8e4
DR = mybir.MatmulPerfMode.DoubleRow
W1SCALE = 16.0   # W1 pre-scaled into fp8 normal range; undone in tanh scale
HSCALE = 16.0    # relu output pre-scaled into fp8 range
W4SCALE = 16.0   # W4 pre-scaled; HSCALE*W4SCALE undone in the conc copy
AF = mybir.ActivationFunctionType
OP = mybir.AluOpType

E = 512          # embed dim
O = 64           # oscillators
DT = 0.01
STEPS = 10       # reference step count
KSTEPS = 1       # kernel uses a single step at 10*DT (adds ~2e-5 rel err)
NCORES = 8
TOK = 4096       # tokens per core (one batch)
P = 128          # partitions / tokens per tile
NT = TOK // P    # 32 tiles per core
TPS = 8          # tiles per superblock
NSB = NT // TPS  # 4 superblocks
NPR = TPS // 2   # tile pairs per superblock
SBC = TPS * O    # phase columns per superblock = 512
HALF_PI = float(np.pi / 2)


def _bcast_ap(ap2d, n):
    """[P, G] -> [P, G, n] view with a step-0 innermost dim (free-dim bcast)."""
    return bass.AP(tensor=ap2d.tensor, offset=ap2d.offset, ap=[*ap2d.ap, [0, n]])


def _split_excess_waits(nc):
    """This toolchain's walrus allows at most 1 sync wait per ordinary
    instruction (2 on EventSemaphore). Hoist excess waits into same-engine
    EventSemaphore instructions inserted just before the offending
    instruction (waits are ANDed, so this is equivalent)."""
    import bass_rust as _br
    n = 0
    for f in nc.m.functions:
        for bb in f.blocks:
            old = bb.instructions
            new = []
            changed = False
            for inst in old:
                si = inst.sync_info
                waits = list(si.on_wait) if (si and si.on_wait) else []
                if len(waits) > 1:
                    changed = True
                    excess, waits = waits[:-1], waits[-1:]
                    while excess:
                        take, excess = excess[:2], excess[2:]
                        es = _br.InstEventSemaphore(name=f"wsplit_{n}")
                        n += 1
                        es.engine = inst.engine
                        es.sync_info = mybir.SyncInfo(on_wait=take, on_update=[])
                        new.append(es)
                    inst.sync_info = mybir.SyncInfo(
                        on_wait=waits,
                        on_update=list(si.on_update) if si.on_update else [])
                new.append(inst)
            if changed:
                bb.instructions = new
    return n


def _build(cdt, has_b2, has_b3, has_b4):
    nc = bass.Bass("TRN2", target_bir_lowering=False, debug=False,
                   enable_asserts=False)
    XT = nc.dram_tensor("xt", [4, P, TOK], FP8, kind="ExternalInput")
    NZ = nc.dram_tensor("noise", [TOK, E], BF16, kind="ExternalInput")
    W1 = nc.dram_tensor("w1", [P, 4, O], FP8, kind="ExternalInput")
    W2 = nc.dram_tensor("w2", [O, O], BF16, kind="ExternalInput")
    W3 = nc.dram_tensor("w3", [O, E], BF16, kind="ExternalInput")
    W4 = nc.dram_tensor("w4", [P, 4, E], FP8, kind="ExternalInput")
    B1 = nc.dram_tensor("b1", [O, 1], F32, kind="ExternalInput")
    OMG = nc.dram_tensor("omgrow", [1, SBC], BF16, kind="ExternalInput")
    FM = nc.dram_tensor("fm", [E], F32, kind="ExternalInput")
    B2R = nc.dram_tensor("b2row", [1, SBC], BF16, kind="ExternalInput")
    B3R = nc.dram_tensor("b3row", [1, E], BF16, kind="ExternalInput")
    B4R = nc.dram_tensor("b4row", [1, E], BF16, kind="ExternalInput")
    # channel-plane layout [4, TOK, E]; host transposes to [TOK, E, 4]
    # during the gather. Keeps every on-device write contiguous. bf16
    # (upconverted on host): halves the dominant DMA traffic; adds
    # ~1e-3 rel err against the 2e-2 gate.
    OUT = nc.dram_tensor("out", [4, TOK, E], BF16, kind="ExternalOutput")

    last_dmas = []      # tail-ladder candidates (walrus sync-wait cap)
    last_eng = {}

    def D(inst):
        last_dmas.append(inst)
        return inst

    def EG(key, inst):
        last_eng[key] = inst
        return inst

    def plane_dram(k, t0):
        """DRAM view of plane k, tiles t0,t0+1 as [P, 2, E]."""
        return OUT[k, t0 * P:(t0 + 2) * P, :].rearrange(
            "(i p) e -> p i e", i=2)

    from contextlib import ExitStack
    with tile.TileContext(nc) as tc, ExitStack() as ctx:
        wp = ctx.enter_context(tc.tile_pool(name="w", bufs=1))
        p0s = ctx.enter_context(tc.tile_pool(name="p0s", bufs=2))
        sp = ctx.enter_context(tc.tile_pool(name="sp", bufs=3))
        mp = ctx.enter_context(tc.tile_pool(name="mp", bufs=3))
        pf = ctx.enter_context(tc.tile_pool(name="pf", bufs=4))
        pts = ctx.enter_context(tc.tile_pool(name="pts", bufs=2))
        h3p = ctx.enter_context(tc.tile_pool(name="h3p", bufs=2))
        ccp = ctx.enter_context(tc.tile_pool(name="ccp", bufs=3))
        nfp = ctx.enter_context(tc.tile_pool(name="nfp", bufs=3))
        ps1 = ctx.enter_context(tc.tile_pool(name="ps1", bufs=2, space="PSUM"))
        ps2 = ctx.enter_context(tc.tile_pool(name="ps2", bufs=2, space="PSUM"))
        php = ctx.enter_context(tc.tile_pool(name="php", bufs=2, space="PSUM"))

        # ---- prefetch + constants. A(0) needs only x0 + w1s (+b1c for the
        # tanh), so those issue FIRST on their queues; everything else
        # follows. One tile per superblock so dependency tracking is exact
        # (a whole-tensor tile would stall A(0) on the full prefetch);
        # chunks spread across the three DMA issue queues so descriptors
        # are outstanding immediately. ----
        SBT = TPS * P  # tokens per superblock
        xsb, nzsb = [], []
        for sb in range(NSB):
            xsb.append(wp.tile([P, 4, SBT], FP8, name=f"xsb{sb}"))
            nzsb.append(wp.tile([P, TPS, E], BF16, name=f"nzsb{sb}"))
        # scalar issues ONLY tiny const DMAs (a bulk DMA here would hit DGE
        # queue backpressure and block the ACT instruction stream for ~15us)
        w1s = wp.tile([P, 4, O], FP8)
        D(nc.scalar.dma_start(out=w1s, in_=W1[:, :, :]))
        b1c = wp.tile([O, 1], F32)
        D(nc.scalar.dma_start(out=b1c, in_=B1[:, :]))
        w2s = wp.tile([O, O], BF16)
        D(nc.scalar.dma_start(out=w2s, in_=W2[:, :]))
        w3s = wp.tile([O, E], BF16)
        D(nc.scalar.dma_start(out=w3s, in_=W3[:, :]))
        w4s = wp.tile([P, 4, E], FP8)
        D(nc.scalar.dma_start(out=w4s, in_=W4[:, :, :]))
        # gpsimd: local compute + tiny consts first (its queue backs up
        # once the bulk nz prefetch below is enqueued), then nz chunks
        ident = wp.tile([P, P], F32)
        make_identity(nc, ident)
        identb = wp.tile([P, P], BF16)
        EG("pool", nc.gpsimd.tensor_copy(out=identb, in_=ident))
        ones = wp.tile([1, 2 * P], BF16)
        EG("dve", nc.vector.memset(ones, 1.0))
        onesr = ones[:, :P]
        halfpi = wp.tile([P, 1], F32)
        EG("dve", nc.vector.memset(halfpi, HALF_PI))
        # warm the ACT function table at t=0 on dummy data: the 1.3us
        # ACT_TABLE_LOAD otherwise lands inside the first tanh, on the
        # startup critical path
        warm = wp.tile([P, 1], F32)
        EG("act", nc.scalar.activation(
            out=warm, in_=halfpi, func=AF.Tanh, bias=0.0, scale=1.0))
        warm2 = wp.tile([P, 1], F32)
        EG("act", nc.scalar.activation(
            out=warm2, in_=halfpi, func=AF.Sin, bias=0.0, scale=1.0))
        b2r = wp.tile([1, SBC], BF16)
        D(nc.gpsimd.dma_start(out=b2r, in_=B2R[:, :]))
        b3r = wp.tile([1, E], BF16)
        D(nc.gpsimd.dma_start(out=b3r, in_=B3R[:, :]))
        b4r = wp.tile([1, E], BF16)
        D(nc.gpsimd.dma_start(out=b4r, in_=B4R[:, :]))
        fmb = wp.tile([P, E], F32)
        fm_bcast = bass.AP(tensor=FM.ap().tensor, offset=0, ap=[[0, P], [1, E]])
        D(nc.gpsimd.dma_start(out=fmb, in_=fm_bcast))
        omgr = wp.tile([1, SBC], BF16)
        D(nc.gpsimd.dma_start(out=omgr, in_=OMG[:, :]))
        # bulk prefetch: x chunks on sync (no compute to block), nz on
        # gpsimd (its next real work is C(0)'s nf DMAs, well after drain)
        for sb in range(NSB):
            D(nc.sync.dma_start(
                out=xsb[sb],
                in_=XT[:, :, sb * SBT:(sb + 1) * SBT]
                .rearrange("c p t -> p c t")))
        for sb in range(NSB):
            D(nc.gpsimd.dma_start(
                out=nzsb[sb],
                in_=NZ[sb * SBT:(sb + 1) * SBT, :]
                .rearrange("(i p) e -> p i e", i=TPS)))

        phs_final = [None] * NSB  # SBUF bf16 phases per superblock

        def phase_a(sb):
            """MLP1 for superblock sb -> ph (PSUM, [P, SBC] f32)."""
            ph = php.tile([P, SBC], F32)
            for pr in range(NPR):
                g0 = 2 * pr
                p0 = ps2.tile([O, 2 * P], F32, tag="ps2")
                # fp8 DoubleRow: two K-chunks per matmul (2 rows/cycle)
                for g in range(2):
                    EG("pe", nc.tensor.matmul(
                        p0, w1s[:, 2 * g:2 * g + 2, :],
                        xsb[sb][:, 2 * g:2 * g + 2, g0 * P:(g0 + 2) * P],
                        start=(g == 0), stop=(g == 1), perf_mode=DR))
                p0t = p0s.tile([O, 2 * P], BF16)
                EG("act", nc.scalar.activation(
                    out=p0t, in_=p0, func=AF.Tanh, bias=b1c,
                    scale=1.0 / W1SCALE))
                # start only on g==0: start=True clears has_written for the
                # WHOLE bank, which would let later group writes clobber
                # groups written before the last start.
                for i in range(2):
                    g = 2 * pr + i
                    EG("pe", nc.tensor.matmul(
                        ph[:, g * O:(g + 1) * O],
                        p0t[:, i * P:(i + 1) * P], w2s[:],
                        start=(g == 0), stop=not has_b2,
                        skip_group_check=True))
                    if has_b2:
                        EG("pe", nc.tensor.matmul(
                            ph[:, g * O:(g + 1) * O], onesr,
                            b2r[:, g * O:(g + 1) * O],
                            start=False, stop=True, skip_group_check=True))
            # pre-add h*omega on the PE (trig then evaluates at ph0+h*omega;
            # shifts the Euler evaluation point, error O(h^2*omega) — same
            # order as the step error itself, ~1e-5 rel)
            EG("pe", nc.tensor.matmul(
                ph, onesr, omgr[:], start=False, stop=True,
                skip_group_check=True))
            return ph

        def phase_b(sb, ph):
            """Single Kuramoto step (h*omega already folded into ph by
            phase A). Trig on ACT, reduce/STT on DVE; the final add runs
            as a PE identity-matmul accumulate + ACT copy to SBUF — the
            chain tail then sits on the PE's own stream instead of
            queueing behind heavy conc/nf ops on the DVE (which measurably
            delayed phs and stalled the next C phase)."""
            s = sp.tile([P, SBC], F32, tag="s")
            EG("act", nc.scalar.activation(
                out=s, in_=ph, func=AF.Sin, bias=0.0, scale=1.0))
            cs = sp.tile([P, SBC], F32, tag="c")
            EG("act", nc.scalar.activation(
                out=cs, in_=ph, func=AF.Sin, bias=halfpi, scale=1.0))
            msum = mp.tile([P, TPS], F32)
            s3 = s[:].rearrange("p (g o) -> p g o", o=O)
            EG("dve", nc.vector.tensor_reduce(
                out=msum, in_=s3, axis=mybir.AxisListType.X, op=OP.add))
            u = sp.tile([P, SBC], BF16, tag="u")
            u3 = u[:].rearrange("p (g o) -> p g o", o=O)
            c3 = cs[:].rearrange("p (g o) -> p g o", o=O)
            EG("dve", nc.vector.scalar_tensor_tensor(
                out=u3, in0=_bcast_ap(msum[:], O), scalar=cdt, in1=c3,
                op0=OP.mult, op1=OP.mult))
            EG("pe", nc.tensor.matmul(
                ph, identb[:], u[:], start=False, stop=True,
                skip_group_check=True))
            phs = pf.tile([P, SBC], BF16)
            EG("act", nc.scalar.copy(out=phs, in_=ph))
            phs_final[sb] = phs[:]

        def phase_c(sb, prs):
            """MLP2 + quaternion assembly + out DMA for superblock sb.
            Two-stage issue over the given pairs: all transp/W3/relu first,
            then all W4/assembly — pair b's W3 fills pair a's relu window
            on the PE (the relu->W4 serial hop was a steady 1.2us gap)."""
            phf = phs_final[sb]
            h3ss = {}
            for pr in prs:
                g0 = 2 * pr
                phT = ps2.tile([O, 2 * P], BF16, tag="ps2")
                for i in range(2):
                    EG("pe", nc.tensor.transpose(
                        phT[:, i * P:(i + 1) * P],
                        phf[:, (g0 + i) * O:(g0 + i + 1) * O],
                        identb))
                phTs = pts.tile([O, 2 * P], BF16)
                EG("act", nc.scalar.copy(out=phTs, in_=phT))
                h3 = ps1.tile([P, 2 * E], F32, tag="ps1")
                for c in range(4):
                    EG("pe", nc.tensor.matmul(
                        h3[:, c * 2 * P:(c + 1) * 2 * P],
                        w3s[:, c * P:(c + 1) * P], phTs[:],
                        start=True, stop=not has_b3))
                    if has_b3:
                        EG("pe", nc.tensor.matmul(
                            h3[:, c * 2 * P:(c + 1) * 2 * P],
                            b3r[:, c * P:(c + 1) * P], ones[:],
                            start=False, stop=True, skip_group_check=True))
                h3s = h3p.tile([P, 2 * E], FP8)
                EG("act", nc.scalar.activation(
                    out=h3s, in_=h3, func=AF.Relu, bias=0.0, scale=HSCALE))
                h3ss[pr] = h3s
            for pr in prs:
                g0 = 2 * pr
                t0 = sb * TPS + g0
                o4 = ps1.tile([P, 2 * E], F32, tag="ps1")
                o42 = o4[:].rearrange("p (i e) -> p i e", i=2)
                h3s4 = h3ss[pr][:].rearrange(
                    "p (c i t) -> p c i t", i=2, t=P)
                for i in range(2):
                    # fp8 DoubleRow: two K-chunks per matmul
                    for g in range(2):
                        EG("pe", nc.tensor.matmul(
                            o42[:, i, :],
                            h3s4[:, 2 * g:2 * g + 2, i, :],
                            w4s[:, 2 * g:2 * g + 2, :],
                            start=(g == 0), stop=(g == 1 and not has_b4),
                            perf_mode=DR))
                    if has_b4:
                        EG("pe", nc.tensor.matmul(
                            o42[:, i, :], onesr, b4r[:],
                            start=False, stop=True, skip_group_check=True))
                # contiguous copies/DMAs only: conc plane from a PSUM->SBUF
                # copy (DVE only — keeping it off ACT shortens the ACT run
                # inside the per-pair chain that gates W3(pr+1) on the PE),
                # noise*fm computed once and DMAed to 3 planes.
                concs = ccp.tile([P, 2, E], BF16)
                EG("dve", nc.vector.tensor_scalar_mul(
                    concs, o42, 1.0 / (HSCALE * W4SCALE)))
                nf = nfp.tile([P, 2, E], BF16)
                fmb2 = bass.AP(tensor=fmb[:].tensor, offset=fmb[:].offset,
                               ap=[fmb[:].ap[0], [0, 2], fmb[:].ap[1]])
                EG("dve", nc.vector.tensor_tensor(
                    out=nf, in0=nzsb[sb][:, g0:g0 + 2, :],
                    in1=fmb2, op=OP.mult))
                D(nc.sync.dma_start(out=plane_dram(0, t0), in_=concs))
                for k in range(1, 4):
                    D(nc.gpsimd.dma_start(out=plane_dram(k, t0), in_=nf))

        # ---- software pipeline: A(it), C(it-1) first half, B(it),
        # C(it-1) second half. Splitting C around B halves the backlog of
        # heavy (1.2us) conc/nf DVE ops queued ahead of B's reduce/STT —
        # the measured blocker of the PE's chain-tail add. ----
        for it in range(NSB + 1):
            ph = phase_a(it) if it < NSB else None
            if it == 0:
                phase_b(0, ph)
                continue
            phase_c(it - 1, range(NPR // 2))
            if it < NSB:
                phase_b(it, ph)
            phase_c(it - 1, range(NPR // 2, NPR))

        # tail ladder: spread end-of-kernel sem waits across SP nops so the
        # final TileContext drain never needs >2 sync waits (walrus cap).
        tail = list(last_eng.values()) + last_dmas[-12:]
        for inst in tail:
            nop = nc.sync.nop()
            add_dep_helper(nop.ins, inst.ins, True, "tail ladder")

    _split_excess_waits(nc)
    return nc


_CACHE = {}


def kernel(x, noise, W1, b1, W2, b2, W3, b3, W4, b4, omega, K, alpha):
    x = np.asarray(x, dtype=np.float32)
    noise = np.asarray(noise, dtype=np.float32)
    W1 = np.asarray(W1, dtype=np.float32)
    W2 = np.asarray(W2, dtype=np.float32)
    W3 = np.asarray(W3, dtype=np.float32)
    W4 = np.asarray(W4, dtype=np.float32)
    b1 = np.asarray(b1, dtype=np.float32)
    b2 = np.asarray(b2, dtype=np.float32)
    b3 = np.asarray(b3, dtype=np.float32)
    b4 = np.asarray(b4, dtype=np.float32)
    omega = np.asarray(omega, dtype=np.float32)
    Kf = float(np.asarray(K))
    alphaf = float(np.asarray(alpha))

    B, S, Ein = x.shape
    assert (B, S, Ein) == (NCORES, TOK, E)

    h = DT * STEPS / KSTEPS       # 2 long Euler steps
    cdt = Kf * h / O
    has_b2 = bool(np.any(b2))
    has_b3 = bool(np.any(b3))
    has_b4 = bool(np.any(b4))
    key = (cdt, has_b2, has_b3, has_b4)
    if key not in _CACHE:
        _CACHE[key] = _build(*key)
    nc = _CACHE[key]

    # host-side prep of tiny params
    w1s = np.ascontiguousarray(
        (W1 * W1SCALE).reshape(4, P, O).transpose(1, 0, 2)
    ).astype(ml_dtypes.float8_e4m3fn)
    w4s = np.ascontiguousarray(
        (W4 * W4SCALE).reshape(4, P, E).transpose(1, 0, 2)
    ).astype(ml_dtypes.float8_e4m3fn)
    b1c = np.ascontiguousarray(b1.reshape(O, 1))
    omgrow = np.ascontiguousarray(
        np.tile(h * omega, TPS).reshape(1, SBC)).astype(ml_dtypes.bfloat16)
    fm = np.sin(alphaf * np.arange(E, dtype=np.float32)).astype(np.float32)
    b2row = np.ascontiguousarray(np.tile(b2, TPS).reshape(1, SBC)).astype(ml_dtypes.bfloat16)
    b3row = np.ascontiguousarray(b3.reshape(1, E)).astype(ml_dtypes.bfloat16)
    # o4 PSUM holds HSCALE*W4SCALE*conc, so a b4 add must be pre-scaled
    b4row = np.ascontiguousarray(
        (b4 * HSCALE * W4SCALE).reshape(1, E)).astype(ml_dtypes.bfloat16)

    in_maps = []
    for i in range(NCORES):
        xt_i = np.ascontiguousarray(
            x[i].T.reshape(4, P, TOK)).astype(ml_dtypes.float8_e4m3fn)
        in_maps.append({
            "xt": xt_i,
            "noise": np.ascontiguousarray(noise[i]).astype(ml_dtypes.bfloat16),
            "w1": w1s, "w2": W2.astype(ml_dtypes.bfloat16),
            "w3": W3.astype(ml_dtypes.bfloat16), "w4": w4s,
            "b1": b1c, "omgrow": omgrow, "fm": fm,
            "b2row": b2row, "b3row": b3row, "b4row": b4row,
        })

    res = run_bass_kernel_spmd(nc, in_maps, core_ids=list(range(NCORES)))
    out = np.empty((B, S, E, 4), dtype=np.float32)
    for i in range(NCORES):
        # device writes bf16 channel planes [4, S, E]; unshard
        # transposes and upconverts
        r = np.asarray(res.results[i]["out"]).reshape(4, S, E)
        out[i] = r.transpose(1, 2, 0).astype(np.float32)
    return out


# revision 77
# speedup vs baseline: 1.0816x; 1.0816x over previous
"""Trainium2 Bass kernel for nn_CognitiveProcessor.

Reference computation (per token, E=512, O=64):
  ph0   = tanh(x @ W1 + b1) @ W2 + b2                  [B,S,O]
  10 Euler steps: ph += DT*(omega + K*mean(sin(ph))*cos(ph))
  conc  = relu(ph @ W3 + b3) @ W4 + b4                 [B,S,E]
  out   = concat([conc, noise*fm, noise*fm, noise*fm], -1)  [B,S,E,4]
  with fm = sin(alpha*arange(E))

Sharding: pure data parallel over batch (B=8 -> 1 batch per core).

Perf notes (216us baseline -> ~100us):
- Output is written as bf16 channel PLANES [4, TOK, E] (conc plane +
  one noise*fm plane DMAed to 3 destinations); the host transposes/
  upconverts to [TOK, E, 4] f32 during the gather. This keeps every
  on-device write contiguous (interleaved [e,k] assembly cost ~4ns/elem
  on strided engine writes) and halves output DMA bytes. Total DMA =
  x fp8 2.15MB + noise bf16 4.3MB + out bf16 16.75MB ~= 24MB at
  360GB/s aggregate (16 engines x 22.5GB/s).
- MLP1 runs in fp8 (x and W1, W1 pre-scaled x16, undone in the tanh
  scale) with DoubleRow perf mode (2 K-rows/cycle); MLP2's big W4
  matmul likewise fp8 DoubleRow (relu emits fp8 scaled x16, W4 scaled
  x16, both undone in the conc copy scale). W3/W2 stay bf16. Adds
  ~1e-3 rel err against the 2e-2 gate (total ~3.7e-3).
- Kuramoto: single Euler step at 10*DT with h*omega pre-added by the
  PE during phase A (trig evaluates at ph0+h*omega; same-order error,
  ~2e-5 rel). Trig on ACT, reduce/STT on DVE; the final add is a PE
  identity-matmul accumulate + ACT copy so the chain tail rides the
  PE's own stream instead of queueing behind conc/nf ops on the DVE.
- Software pipeline per iteration: A(sb) MLP1, C(sb-1) MLP2+out-DMA,
  B(sb) Kuramoto; all inputs prefetched to SBUF upfront (per-SB tiles
  so dependency tracking is exact). Bulk prefetch must NOT issue on
  the scalar queue: DGE backpressure blocks the ACT instruction
  stream ~15us. conc copies alternate ACT/DVE; out-DMAs ride the
  gpsimd + sync queues.
"""

import numpy as np
import ml_dtypes

import concourse.bass as bass
import concourse.tile as tile
from concourse import mybir
from concourse.tile import add_dep_helper
from concourse.bass_utils import run_bass_kernel_spmd
from concourse.masks import make_identity

F32 = mybir.dt.float32
BF16 = mybir.dt.bfloat16
FP8 = mybir.dt.float8e4
DR = mybir.MatmulPerfMode.DoubleRow
W1SCALE = 16.0   # W1 pre-scaled into fp8 normal range; undone in tanh scale
HSCALE = 16.0    # relu output pre-scaled into fp8 range
W4SCALE = 16.0   # W4 pre-scaled; HSCALE*W4SCALE undone in the conc copy
AF = mybir.ActivationFunctionType
OP = mybir.AluOpType

E = 512          # embed dim
O = 64           # oscillators
DT = 0.01
STEPS = 10       # reference step count
KSTEPS = 1       # kernel uses a single step at 10*DT (adds ~2e-5 rel err)
NCORES = 8
TOK = 4096       # tokens per core (one batch)
P = 128          # partitions / tokens per tile
NT = TOK // P    # 32 tiles per core
TPS = 8          # tiles per superblock
NSB = NT // TPS  # 4 superblocks
NPR = TPS // 2   # tile pairs per superblock
SBC = TPS * O    # phase columns per superblock = 512
HALF_PI = float(np.pi / 2)


def _bcast_ap(ap2d, n):
    """[P, G] -> [P, G, n] view with a step-0 innermost dim (free-dim bcast)."""
    return bass.AP(tensor=ap2d.tensor, offset=ap2d.offset, ap=[*ap2d.ap, [0, n]])


def _split_excess_waits(nc):
    """This toolchain's walrus allows at most 1 sync wait per ordinary
    instruction (2 on EventSemaphore). Hoist excess waits into same-engine
    EventSemaphore instructions inserted just before the offending
    instruction (waits are ANDed, so this is equivalent)."""
    import bass_rust as _br
    n = 0
    for f in nc.m.functions:
        for bb in f.blocks:
            old = bb.instructions
            new = []
            changed = False
            for inst in old:
                si = inst.sync_info
                waits = list(si.on_wait) if (si and si.on_wait) else []
                if len(waits) > 1:
                    changed = True
                    excess, waits = waits[:-1], waits[-1:]
                    while excess:
                        take, excess = excess[:2], excess[2:]
                        es = _br.InstEventSemaphore(name=f"wsplit_{n}")
                        n += 1
                        es.engine = inst.engine
                        es.sync_info = mybir.SyncInfo(on_wait=take, on_update=[])
                        new.append(es)
                    inst.sync_info = mybir.SyncInfo(
                        on_wait=waits,
                        on_update=list(si.on_update) if si.on_update else [])
                new.append(inst)
            if changed:
                bb.instructions = new
    return n


def _build(cdt, has_b2, has_b3, has_b4):
    nc = bass.Bass("TRN2", target_bir_lowering=False, debug=False,
                   enable_asserts=False)
    XT = nc.dram_tensor("xt", [4, P, TOK], FP8, kind="ExternalInput")
    NZ = nc.dram_tensor("noise", [TOK, E], BF16, kind="ExternalInput")
    W1 = nc.dram_tensor("w1", [P, 4, O], FP8, kind="ExternalInput")
    W2 = nc.dram_tensor("w2", [O, O], BF16, kind="ExternalInput")
    W3 = nc.dram_tensor("w3", [O, E], BF16, kind="ExternalInput")
    W4 = nc.dram_tensor("w4", [P, 4, E], FP8, kind="ExternalInput")
    B1 = nc.dram_tensor("b1", [O, 1], F32, kind="ExternalInput")
    OMG = nc.dram_tensor("omgrow", [1, SBC], BF16, kind="ExternalInput")
    FM = nc.dram_tensor("fm", [E], F32, kind="ExternalInput")
    B2R = nc.dram_tensor("b2row", [1, SBC], BF16, kind="ExternalInput")
    B3R = nc.dram_tensor("b3row", [1, E], BF16, kind="ExternalInput")
    B4R = nc.dram_tensor("b4row", [1, E], BF16, kind="ExternalInput")
    # channel-plane layout [4, TOK, E]; host transposes to [TOK, E, 4]
    # during the gather. Keeps every on-device write contiguous. bf16
    # (upconverted on host): halves the dominant DMA traffic; adds
    # ~1e-3 rel err against the 2e-2 gate.
    OUT = nc.dram_tensor("out", [4, TOK, E], BF16, kind="ExternalOutput")

    last_dmas = []      # tail-ladder candidates (walrus sync-wait cap)
    last_eng = {}

    def D(inst):
        last_dmas.append(inst)
        return inst

    def EG(key, inst):
        last_eng[key] = inst
        return inst

    def plane_dram(k, t0):
        """DRAM view of plane k, tiles t0,t0+1 as [P, 2, E]."""
        return OUT[k, t0 * P:(t0 + 2) * P, :].rearrange(
            "(i p) e -> p i e", i=2)

    from contextlib import ExitStack
    with tile.TileContext(nc) as tc, ExitStack() as ctx:
        wp = ctx.enter_context(tc.tile_pool(name="w", bufs=1))
        p0s = ctx.enter_context(tc.tile_pool(name="p0s", bufs=2))
        sp = ctx.enter_context(tc.tile_pool(name="sp", bufs=3))
        mp = ctx.enter_context(tc.tile_pool(name="mp", bufs=3))
        pf = ctx.enter_context(tc.tile_pool(name="pf", bufs=4))
        pts = ctx.enter_context(tc.tile_pool(name="pts", bufs=2))
        h3p = ctx.enter_context(tc.tile_pool(name="h3p", bufs=2))
        ccp = ctx.enter_context(tc.tile_pool(name="ccp", bufs=3))
        nfp = ctx.enter_context(tc.tile_pool(name="nfp", bufs=3))
        ps1 = ctx.enter_context(tc.tile_pool(name="ps1", bufs=2, space="PSUM"))
        ps2 = ctx.enter_context(tc.tile_pool(name="ps2", bufs=2, space="PSUM"))
        php = ctx.enter_context(tc.tile_pool(name="php", bufs=2, space="PSUM"))

        # ---- prefetch + constants. A(0) needs only x0 + w1s (+b1c for the
        # tanh), so those issue FIRST on their queues; everything else
        # follows. One tile per superblock so dependency tracking is exact
        # (a whole-tensor tile would stall A(0) on the full prefetch);
        # chunks spread across the three DMA issue queues so descriptors
        # are outstanding immediately. ----
        SBT = TPS * P  # tokens per superblock
        xsb, nzsb = [], []
        for sb in range(NSB):
            xsb.append(wp.tile([P, 4, SBT], FP8, name=f"xsb{sb}"))
            nzsb.append(wp.tile([P, TPS, E], BF16, name=f"nzsb{sb}"))
        # scalar issues ONLY tiny const DMAs (a bulk DMA here would hit DGE
        # queue backpressure and block the ACT instruction stream for ~15us)
        w1s = wp.tile([P, 4, O], FP8)
        D(nc.scalar.dma_start(out=w1s, in_=W1[:, :, :]))
        b1c = wp.tile([O, 1], F32)
        D(nc.scalar.dma_start(out=b1c, in_=B1[:, :]))
        w2s = wp.tile([O, O], BF16)
        D(nc.scalar.dma_start(out=w2s, in_=W2[:, :]))
        w3s = wp.tile([O, E], BF16)
        D(nc.scalar.dma_start(out=w3s, in_=W3[:, :]))
        w4s = wp.tile([P, 4, E], FP8)
        D(nc.scalar.dma_start(out=w4s, in_=W4[:, :, :]))
        # gpsimd: local compute + tiny consts first (its queue backs up
        # once the bulk nz prefetch below is enqueued), then nz chunks
        ident = wp.tile([P, P], F32)
        make_identity(nc, ident)
        identb = wp.tile([P, P], BF16)
        EG("pool", nc.gpsimd.tensor_copy(out=identb, in_=ident))
        ones = wp.tile([1, 2 * P], BF16)
        EG("dve", nc.vector.memset(ones, 1.0))
        onesr = ones[:, :P]
        halfpi = wp.tile([P, 1], F32)
        EG("dve", nc.vector.memset(halfpi, HALF_PI))
        # warm the ACT function table at t=0 on dummy data: the 1.3us
        # ACT_TABLE_LOAD otherwise lands inside the first tanh, on the
        # startup critical path
        warm = wp.tile([P, 1], F32)
        EG("act", nc.scalar.activation(
            out=warm, in_=halfpi, func=AF.Tanh, bias=0.0, scale=1.0))
        warm2 = wp.tile([P, 1], F32)
        EG("act", nc.scalar.activation(
            out=warm2, in_=halfpi, func=AF.Sin, bias=0.0, scale=1.0))
        b2r = wp.tile([1, SBC], BF16)
        D(nc.gpsimd.dma_start(out=b2r, in_=B2R[:, :]))
        b3r = wp.tile([1, E], BF16)
        D(nc.gpsimd.dma_start(out=b3r, in_=B3R[:, :]))
        b4r = wp.tile([1, E], BF16)
        D(nc.gpsimd.dma_start(out=b4r, in_=B4R[:, :]))
        fmb = wp.tile([P, E], F32)
        fm_bcast = bass.AP(tensor=FM.ap().tensor, offset=0, ap=[[0, P], [1, E]])
        D(nc.gpsimd.dma_start(out=fmb, in_=fm_bcast))
        omgr = wp.tile([1, SBC], BF16)
        D(nc.gpsimd.dma_start(out=omgr, in_=OMG[:, :]))
        # bulk prefetch: x chunks on sync (no compute to block), nz on
        # gpsimd (its next real work is C(0)'s nf DMAs, well after drain)
        for sb in range(NSB):
            D(nc.sync.dma_start(
                out=xsb[sb],
                in_=XT[:, :, sb * SBT:(sb + 1) * SBT]
                .rearrange("c p t -> p c t")))
        for sb in range(NSB):
            D(nc.gpsimd.dma_start(
                out=nzsb[sb],
                in_=NZ[sb * SBT:(sb + 1) * SBT, :]
                .rearrange("(i p) e -> p i e", i=TPS)))

        phs_final = [None] * NSB  # SBUF bf16 phases per superblock

        def phase_a(sb):
            """MLP1 for superblock sb -> ph (PSUM, [P, SBC] f32)."""
            ph = php.tile([P, SBC], F32)
            for pr in range(NPR):
                g0 = 2 * pr
                p0 = ps2.tile([O, 2 * P], F32, tag="ps2")
                # fp8 DoubleRow: two K-chunks per matmul (2 rows/cycle)
                for g in range(2):
                    EG("pe", nc.tensor.matmul(
                        p0, w1s[:, 2 * g:2 * g + 2, :],
                        xsb[sb][:, 2 * g:2 * g + 2, g0 * P:(g0 + 2) * P],
                        start=(g == 0), stop=(g == 1), perf_mode=DR))
                p0t = p0s.tile([O, 2 * P], BF16)
                EG("act", nc.scalar.activation(
                    out=p0t, in_=p0, func=AF.Tanh, bias=b1c,
                    scale=1.0 / W1SCALE))
                # start only on g==0: start=True clears has_written for the
                # WHOLE bank, which would let later group writes clobber
                # groups written before the last start.
                for i in range(2):
                    g = 2 * pr + i
                    EG("pe", nc.tensor.matmul(
                        ph[:, g * O:(g + 1) * O],
                        p0t[:, i * P:(i + 1) * P], w2s[:],
                        start=(g == 0), stop=not has_b2,
                        skip_group_check=True))
                    if has_b2:
                        EG("pe", nc.tensor.matmul(
                            ph[:, g * O:(g + 1) * O], onesr,
                            b2r[:, g * O:(g + 1) * O],
                            start=False, stop=True, skip_group_check=True))
            # pre-add h*omega on the PE (trig then evaluates at ph0+h*omega;
            # shifts the Euler evaluation point, error O(h^2*omega) — same
            # order as the step error itself, ~1e-5 rel)
            EG("pe", nc.tensor.matmul(
                ph, onesr, omgr[:], start=False, stop=True,
                skip_group_check=True))
            return ph

        def phase_b(sb, ph):
            """Single Kuramoto step (h*omega already folded into ph by
            phase A). Trig on ACT, reduce/STT on DVE; the final add runs
            as a PE identity-matmul accumulate + ACT copy to SBUF — the
            chain tail then sits on the PE's own stream instead of
            queueing behind heavy conc/nf ops on the DVE (which measurably
            delayed phs and stalled the next C phase)."""
            s = sp.tile([P, SBC], F32, tag="s")
            EG("act", nc.scalar.activation(
                out=s, in_=ph, func=AF.Sin, bias=0.0, scale=1.0))
            cs = sp.tile([P, SBC], F32, tag="c")
            EG("act", nc.scalar.activation(
                out=cs, in_=ph, func=AF.Sin, bias=halfpi, scale=1.0))
            msum = mp.tile([P, TPS], F32)
            s3 = s[:].rearrange("p (g o) -> p g o", o=O)
            EG("dve", nc.vector.tensor_reduce(
                out=msum, in_=s3, axis=mybir.AxisListType.X, op=OP.add))
            u = sp.tile([P, SBC], BF16, tag="u")
            u3 = u[:].rearrange("p (g o) -> p g o", o=O)
            c3 = cs[:].rearrange("p (g o) -> p g o", o=O)
            EG("dve", nc.vector.scalar_tensor_tensor(
                out=u3, in0=_bcast_ap(msum[:], O), scalar=cdt, in1=c3,
                op0=OP.mult, op1=OP.mult))
            EG("pe", nc.tensor.matmul(
                ph, identb[:], u[:], start=False, stop=True,
                skip_group_check=True))
            phs = pf.tile([P, SBC], BF16)
            EG("act", nc.scalar.copy(out=phs, in_=ph))
            phs_final[sb] = phs[:]

        def phase_c(sb, prs):
            """MLP2 + quaternion assembly + out DMA for superblock sb."""
            phf = phs_final[sb]
            for pr in prs:
                g0 = 2 * pr
                t0 = sb * TPS + g0
                # noise*fm first: it depends only on prefetched inputs, so
                # issuing its TT + 3 plane-DMAs (75% of this pair's output
                # bytes) ahead of the matmul chain starts the DMA earlier
                # and leaves only the small conc plane for the tail.
                nf = nfp.tile([P, 2, E], BF16)
                fmb2 = bass.AP(tensor=fmb[:].tensor, offset=fmb[:].offset,
                               ap=[fmb[:].ap[0], [0, 2], fmb[:].ap[1]])
                EG("dve", nc.vector.tensor_tensor(
                    out=nf, in0=nzsb[sb][:, g0:g0 + 2, :],
                    in1=fmb2, op=OP.mult))
                for k in range(1, 4):
                    D(nc.gpsimd.dma_start(out=plane_dram(k, t0), in_=nf))
                phT = ps2.tile([O, 2 * P], BF16, tag="ps2")
                for i in range(2):
                    EG("pe", nc.tensor.transpose(
                        phT[:, i * P:(i + 1) * P],
                        phf[:, (g0 + i) * O:(g0 + i + 1) * O],
                        identb))
                phTs = pts.tile([O, 2 * P], BF16)
                EG("act", nc.scalar.copy(out=phTs, in_=phT))
                h3 = ps1.tile([P, 2 * E], F32, tag="ps1")
                for c in range(4):
                    EG("pe", nc.tensor.matmul(
                        h3[:, c * 2 * P:(c + 1) * 2 * P],
                        w3s[:, c * P:(c + 1) * P], phTs[:],
                        start=True, stop=not has_b3))
                    if has_b3:
                        EG("pe", nc.tensor.matmul(
                            h3[:, c * 2 * P:(c + 1) * 2 * P],
                            b3r[:, c * P:(c + 1) * P], ones[:],
                            start=False, stop=True, skip_group_check=True))
                h3s = h3p.tile([P, 2 * E], FP8)
                EG("act", nc.scalar.activation(
                    out=h3s, in_=h3, func=AF.Relu, bias=0.0, scale=HSCALE))

                o4 = ps1.tile([P, 2 * E], F32, tag="ps1")
                o42 = o4[:].rearrange("p (i e) -> p i e", i=2)
                h3s4 = h3s[:].rearrange("p (c i t) -> p c i t", i=2, t=P)
                for i in range(2):
                    # fp8 DoubleRow: two K-chunks per matmul
                    for g in range(2):
                        EG("pe", nc.tensor.matmul(
                            o42[:, i, :],
                            h3s4[:, 2 * g:2 * g + 2, i, :],
                            w4s[:, 2 * g:2 * g + 2, :],
                            start=(g == 0), stop=(g == 1 and not has_b4),
                            perf_mode=DR))
                    if has_b4:
                        EG("pe", nc.tensor.matmul(
                            o42[:, i, :], onesr, b4r[:],
                            start=False, stop=True, skip_group_check=True))
                # conc plane: PSUM->SBUF copy on DVE (off ACT, which gates
                # the per-pair chain), then the last small DMA of the pair.
                concs = ccp.tile([P, 2, E], BF16)
                EG("dve", nc.vector.tensor_scalar_mul(
                    concs, o42, 1.0 / (HSCALE * W4SCALE)))
                D(nc.sync.dma_start(out=plane_dram(0, t0), in_=concs))

        # ---- software pipeline: A(it), C(it-1) first half, B(it),
        # C(it-1) second half. Splitting C around B halves the backlog of
        # heavy (1.2us) conc/nf DVE ops queued ahead of B's reduce/STT —
        # the measured blocker of the PE's chain-tail add. ----
        for it in range(NSB + 1):
            ph = phase_a(it) if it < NSB else None
            if it == 0:
                phase_b(0, ph)
                continue
            phase_c(it - 1, range(NPR // 2))
            if it < NSB:
                phase_b(it, ph)
            phase_c(it - 1, range(NPR // 2, NPR))

        # tail ladder: spread end-of-kernel sem waits across SP nops so the
        # final TileContext drain never needs >2 sync waits (walrus cap).
        tail = list(last_eng.values()) + last_dmas[-12:]
        for inst in tail:
            nop = nc.sync.nop()
            add_dep_helper(nop.ins, inst.ins, True, "tail ladder")

    _split_excess_waits(nc)
    return nc


_CACHE = {}


def kernel(x, noise, W1, b1, W2, b2, W3, b3, W4, b4, omega, K, alpha):
    x = np.asarray(x, dtype=np.float32)
    noise = np.asarray(noise, dtype=np.float32)
    W1 = np.asarray(W1, dtype=np.float32)
    W2 = np.asarray(W2, dtype=np.float32)
    W3 = np.asarray(W3, dtype=np.float32)
    W4 = np.asarray(W4, dtype=np.float32)
    b1 = np.asarray(b1, dtype=np.float32)
    b2 = np.asarray(b2, dtype=np.float32)
    b3 = np.asarray(b3, dtype=np.float32)
    b4 = np.asarray(b4, dtype=np.float32)
    omega = np.asarray(omega, dtype=np.float32)
    Kf = float(np.asarray(K))
    alphaf = float(np.asarray(alpha))

    B, S, Ein = x.shape
    assert (B, S, Ein) == (NCORES, TOK, E)

    h = DT * STEPS / KSTEPS       # 2 long Euler steps
    cdt = Kf * h / O
    has_b2 = bool(np.any(b2))
    has_b3 = bool(np.any(b3))
    has_b4 = bool(np.any(b4))
    key = (cdt, has_b2, has_b3, has_b4)
    if key not in _CACHE:
        _CACHE[key] = _build(*key)
    nc = _CACHE[key]

    # host-side prep of tiny params
    w1s = np.ascontiguousarray(
        (W1 * W1SCALE).reshape(4, P, O).transpose(1, 0, 2)
    ).astype(ml_dtypes.float8_e4m3fn)
    w4s = np.ascontiguousarray(
        (W4 * W4SCALE).reshape(4, P, E).transpose(1, 0, 2)
    ).astype(ml_dtypes.float8_e4m3fn)
    b1c = np.ascontiguousarray(b1.reshape(O, 1))
    omgrow = np.ascontiguousarray(
        np.tile(h * omega, TPS).reshape(1, SBC)).astype(ml_dtypes.bfloat16)
    fm = np.sin(alphaf * np.arange(E, dtype=np.float32)).astype(np.float32)
    b2row = np.ascontiguousarray(np.tile(b2, TPS).reshape(1, SBC)).astype(ml_dtypes.bfloat16)
    b3row = np.ascontiguousarray(b3.reshape(1, E)).astype(ml_dtypes.bfloat16)
    # o4 PSUM holds HSCALE*W4SCALE*conc, so a b4 add must be pre-scaled
    b4row = np.ascontiguousarray(
        (b4 * HSCALE * W4SCALE).reshape(1, E)).astype(ml_dtypes.bfloat16)

    in_maps = []
    for i in range(NCORES):
        xt_i = np.ascontiguousarray(
            x[i].T.reshape(4, P, TOK)).astype(ml_dtypes.float8_e4m3fn)
        in_maps.append({
            "xt": xt_i,
            "noise": np.ascontiguousarray(noise[i]).astype(ml_dtypes.bfloat16),
            "w1": w1s, "w2": W2.astype(ml_dtypes.bfloat16),
            "w3": W3.astype(ml_dtypes.bfloat16), "w4": w4s,
            "b1": b1c, "omgrow": omgrow, "fm": fm,
            "b2row": b2row, "b3row": b3row, "b4row": b4row,
        })

    res = run_bass_kernel_spmd(nc, in_maps, core_ids=list(range(NCORES)))
    out = np.empty((B, S, E, 4), dtype=np.float32)
    for i in range(NCORES):
        # device writes bf16 channel planes [4, S, E]; unshard
        # transposes and upconverts
        r = np.asarray(res.results[i]["out"]).reshape(4, S, E)
        out[i] = r.transpose(1, 2, 0).astype(np.float32)
    return out
